# revision 1
# baseline (speedup 1.0000x reference)
"""Chamfer distance (squared-L2) kernel for 8 Trainium2 NeuronCores.

Problem: xyz1 (4, 8192, 3) f32, xyz2 (4, 8192, 3) f32.
  d[b,n,m] = ||p_n - q_m||^2 ; out = mean_n(min_m d) + mean_m(min_n d)  (scalar f32)

Sharding: 8 cores = 4 batches x 2-way split of N.  Each core handles a
(4096 x 8192) block of the distance matrix: full row-mins for its 4096 rows
plus partial column-mins (later min-combined across the 2 row-shards on host).

Per-core algorithm:
  - PE emits *complete* distance tiles via an augmented matmul:
      d[n,m] = sum_c (-2 p_nc) q_mc + 1*||q_m||^2 + ||p_n||^2 * 1
    fp32 matmul is 4 cyc/row on TRN2, so each fp32 factor is split into
    3 bf16 components (hi/mid/lo); keeping all product terms >= 2^-26
    gives K=24 bf16 rows (exact products accumulated in fp32 PSUM,
    total error ~1e-6) while streaming at 1 col/cycle.
  - ScalarE (ACT) copies PSUM distance tiles to SBUF, narrowing to bf16
    (round-to-nearest noise on the mins averages out over 32k rows/cols).
  - VectorE row-mins: a custom DVE op fuses pairwise min of the two chunk
    halves with a min-accumulate over the free dim — two unit-stride bf16
    streams keep both SBUF read ports busy (~2 elems/cycle/lane).
  - Column-min accumulator (bf16) updated with tensor_tensor(min), which
    runs in the 2x_1P DVE perf mode for bf16 SBUF operands.
    Both DVE passes sit at the 2-read-ports/cycle/lane structural floor.
  - Final: PE transposes the (128, 8192) column-min accumulator in 128x128
    blocks; VectorE does segmented min-reduces to produce per-column mins.
Outputs per core: rowmin (128, 32) f32, colmin (128, 64) f32 -> tiny host
combine (sums / pairwise min) produces the scalar.
"""

import os
import numpy as np
import ml_dtypes

B = 4
N = 8192
M = 8192
NCORES = 8
NLOC = N // 2            # 4096 rows per core
P = 128                  # partitions
NT = NLOC // P           # 32 n-tiles
CHUNK = 2048             # columns per PSUM macro-tile
NCH = M // CHUNK         # 4 chunks
MMF = 512                # matmul free dim (one PSUM bank of fp32)
KAUG = 24                # augmented contraction size (bf16 rows)
NBLK = M // P            # 64 column blocks of 128 for the final fold

BF16 = ml_dtypes.bfloat16

_NC_CACHE = {}
LAST_RESULTS = None


def _register_min_op():
    """Register (once) a custom DVE op: out = min(in0, in1) elementwise,
    accum_out = min(s0, min over free dim of out).  Used for the fused
    half-pair + row-min reduction; the uop table ships inside the NEFF.
    (The native TENSOR_TENSOR_REDUCE opcode faults on this runtime.)
    """
    from concourse import dve_ops
    from concourse.dve_spec import Spec, Src0, Src1, C0, lower, minn
    from concourse.dve_uop import DveOpSpec

    name = "PAIR_MIN_ACCMIN_ANT"
    for o in dve_ops.OPS:
        if o.name == name:
            return o

    def _ref(in0, in1, c0, c1, c2):
        b = np.minimum(in0.astype(np.float32), in1).astype(np.float32)
        return b, np.minimum(
            np.float32(c0), b.reshape(b.shape[0], -1).min(axis=-1, keepdims=True)
        )

    spec = Spec(body=minn(Src0, Src1), accum=minn, accum_init=C0, reference=_ref)
    row = max(dve_ops._SUB_OPCODE_FOR_NAME.values()) + 1
    dve_ops._SUB_OPCODE_FOR_NAME[name] = row
    shas = {}
    for ver in ("v3", "v4"):
        s = DveOpSpec(name=name, opcode=row, uops=lower(spec, ver=ver), rd1_en=True)
        shas[ver] = s.sha(ver)
    op = dve_ops.DveOp(name, spec, subdim=False, uops_sha=shas)
    dve_ops.OPS.append(op)
    dve_ops.CUSTOM_DVE_SPECS[name] = spec
    return op


def _build_nc():
    import concourse.bass as bass
    import concourse.mybir as mybir
    import concourse.tile as tile
    import concourse.bacc as bacc
    from concourse.masks import make_identity
    from contextlib import ExitStack

    min_op = _register_min_op()

    f32 = mybir.dt.float32
    bf16 = mybir.dt.bfloat16
    MIN = mybir.AluOpType.min
    AXX = mybir.AxisListType.X

    nc = bacc.Bacc(trn_type="TRN2")
    a1_d = nc.dram_tensor("aug1", (KAUG, NLOC), bf16, kind="ExternalInput").ap()
    a2_d = nc.dram_tensor("aug2", (KAUG, M), bf16, kind="ExternalInput").ap()
    rowmin_d = nc.dram_tensor("rowmin", (P, NT), f32, kind="ExternalOutput").ap()
    colmin_d = nc.dram_tensor("colmin", (P, NBLK), f32, kind="ExternalOutput").ap()

    with tile.TileContext(nc) as tc, ExitStack() as ctx:
        consts = ctx.enter_context(tc.tile_pool(name="consts", bufs=1))
        accp = ctx.enter_context(tc.tile_pool(name="accp", bufs=1))
        psum = ctx.enter_context(tc.tile_pool(name="psum", bufs=2, space="PSUM"))
        dsb = ctx.enter_context(tc.tile_pool(name="dsb", bufs=3))
        scr = ctx.enter_context(tc.tile_pool(name="scr", bufs=2))
        outp = ctx.enter_context(tc.tile_pool(name="outp", bufs=1))

        # strip-wise input DMAs: the first matmuls only need the first strips,
        # so compute starts while the rest of the operands stream in
        a1s = consts.tile([KAUG, NLOC], bf16)
        a2s = consts.tile([KAUG, M], bf16)
        nc.sync.dma_start(out=a1s[:, :P], in_=a1_d[:, :P])
        for c in range(NCH):
            eng = nc.sync if c % 2 == 0 else nc.gpsimd
            eng.dma_start(
                out=a2s[:, c * CHUNK:(c + 1) * CHUNK],
                in_=a2_d[:, c * CHUNK:(c + 1) * CHUNK],
            )
        nc.gpsimd.dma_start(out=a1s[:, P:], in_=a1_d[:, P:])
        ident = consts.tile([P, P], bf16)
        make_identity(nc, ident)

        # single column-min accumulator, bf16 (DVE tensor_tensor min runs at
        # 2x_1P for bf16 SBUF operands)
        acc = accp.tile([P, M], bf16)

        rmall = outp.tile([P, NT], f32)
        cmall = outp.tile([P, NBLK], f32)

        repeat = int(os.environ.get("CHAMFER_REPEAT", "1"))
        for rep in range(repeat):
          for t in range(NT):
            # one full-width bf16 distance row-block: fewer, larger DVE ops
            # amortize the per-op SBUF access bubble (~58-120 cycles each)
            d = dsb.tile([P, M], bf16, tag="d")
            for c in range(NCH):
                ps = psum.tile([P, CHUNK], f32, tag="ps")
                for j in range(CHUNK // MMF):
                    col = c * CHUNK + j * MMF
                    nc.tensor.matmul(
                        ps[:, j * MMF:(j + 1) * MMF],
                        a1s[:, t * P:(t + 1) * P],
                        a2s[:, col:col + MMF],
                        start=True,
                        stop=True,
                    )
                # ACT copies + narrows to bf16 (min results only need bf16:
                # round-to-nearest noise averages out over 32k rows/cols)
                nc.scalar.copy(out=d[:, c * CHUNK:(c + 1) * CHUNK], in_=ps)

            if (t < 4 or t == NT - 1) and rep == 0:
                # chunk-granular first rows (DVE starts as soon as the first
                # aug2 strip lands) and last row (the final fold's transposes
                # start while the last column-min updates stream)
                PSZ = CHUNK
                NP = M // PSZ
                r0 = scr.tile([P, NP], f32, tag="r0stage", name=f"r0_{t}")
                for pc in range(NP):
                    dslice = d[:, pc * PSZ:(pc + 1) * PSZ]
                    sc0 = scr.tile([P, PSZ // 2], bf16, tag="sc")
                    nc.vector._custom_dve(
                        min_op,
                        out=sc0,
                        in0=dslice[:, : PSZ // 2],
                        in1=dslice[:, PSZ // 2:],
                        s0=1e30,
                        accum_out=r0[:, pc:pc + 1],
                    )
                    accslice = acc[:, pc * PSZ:(pc + 1) * PSZ]
                    if t == 0:
                        nc.vector.tensor_copy(out=accslice, in_=dslice)
                    else:
                        nc.vector.tensor_tensor(
                            out=accslice, in0=dslice, in1=accslice, op=MIN
                        )
                nc.vector.tensor_reduce(
                    out=rmall[:, t:t + 1], in_=r0, axis=AXX, op=MIN
                )
                continue

            # fused half-pairing min + row-min accumulate over the whole row:
            # two unit-stride bf16 streams keep both SBUF read ports busy,
            # accum register writes the exact row-min directly
            sc = scr.tile([P, M // 2], bf16, tag="sc")
            nc.vector._custom_dve(
                min_op,
                out=sc,
                in0=d[:, : M // 2],
                in1=d[:, M // 2:],
                s0=1e30,
                accum_out=rmall[:, t:t + 1],
            )

            # column-min accumulate (bf16 2x_1P mode)
            nc.vector.tensor_tensor(out=acc, in0=d, in1=acc, op=MIN)

        # fold the column-min accumulator over the partition axis:
        # PE-transpose 128x128 bf16 blocks into PSUM, then segmented min-reduce.
        TGRP = 8   # blocks per PSUM tile: finer groups shorten the fold tail
        for g in range(NBLK // TGRP):
            psT = psum.tile([P, TGRP * P], bf16, tag="ps")
            for j in range(TGRP):
                k = g * TGRP + j
                nc.tensor.transpose(
                    psT[:, j * P:(j + 1) * P], acc[:, k * P:(k + 1) * P], ident
                )
            seg = psT.rearrange("p (j x) -> p j x", x=P)
            nc.vector.tensor_reduce(
                out=cmall[:, g * TGRP:(g + 1) * TGRP], in_=seg, axis=AXX, op=MIN
            )

        nc.sync.dma_start(out=rowmin_d, in_=rmall)
        nc.sync.dma_start(out=colmin_d, in_=cmall)
    nc.compile()
    return nc


def _get_nc():
    if "nc" not in _NC_CACHE:
        _NC_CACHE["nc"] = _build_nc()
    return _NC_CACHE["nc"]


def _split3(x64):
    """Split float64 array into 3 bf16 components summing to ~x (rel ~2^-27)."""
    h = x64.astype(BF16)
    r = x64 - h.astype(np.float64)
    m = r.astype(BF16)
    r2 = r - m.astype(np.float64)
    l = r2.astype(BF16)
    return h, m, l


def _make_augs(p, q):
    """Build augmented bf16 operands for one core.

    p: (NLOC, 3) f32 row points, q: (M, 3) f32 column points.
    Returns aug1 (KAUG, NLOC), aug2 (KAUG, M) bf16 such that
    aug1.T @ aug2 ~= squared distance matrix (fp32-accurate).
    """
    p64 = p.astype(np.float64)
    q64 = q.astype(np.float64)
    a = -2.0 * p64                      # lhs coordinate factors
    s1 = (p64 * p64).sum(-1)            # ||p||^2
    s2 = (q64 * q64).sum(-1)            # ||q||^2

    ah, am, al = _split3(a)
    bh, bm, bl = _split3(q64)
    s1h, s1m, s1l = _split3(s1)
    s2h, s2m, s2l = _split3(s2)

    ones_n = np.ones(p.shape[0], BF16)
    ones_m = np.ones(q.shape[0], BF16)

    aug1 = np.empty((KAUG, p.shape[0]), BF16)
    aug2 = np.empty((KAUG, q.shape[0]), BF16)
    r = 0
    for c in range(3):
        pairs = [
            (ah[:, c], bh[:, c]),
            (ah[:, c], bm[:, c]),
            (am[:, c], bh[:, c]),
            (am[:, c], bm[:, c]),
            (ah[:, c], bl[:, c]),
            (al[:, c], bh[:, c]),
        ]
        for u, v in pairs:
            aug1[r] = u
            aug2[r] = v
            r += 1
    for s2x in (s2h, s2m, s2l):
        aug1[r] = ones_n
        aug2[r] = s2x
        r += 1
    for s1x in (s1h, s1m, s1l):
        aug1[r] = s1x
        aug2[r] = ones_m
        r += 1
    assert r == KAUG
    return aug1, aug2


def _get_runner():
    """Build (once) a cached jitted SPMD executor for the bass program.

    Mirrors concourse.bass2jax.run_bass_via_pjrt's multi-core path, but caches
    the jitted callable so repeat kernel() calls skip retrace/recompile.
    """
    if "runner" in _NC_CACHE:
        return _NC_CACHE["runner"]

    import jax
    import concourse.mybir as mybir
    from jax.experimental.shard_map import shard_map
    from jax.sharding import Mesh, PartitionSpec
    from concourse.bass2jax import (
        install_neuronx_cc_hook,
        partition_id_tensor,
        _bass_exec_p,
    )

    install_neuronx_cc_hook()
    nc = _get_nc()

    in_names, out_names, out_avals, zero_outs = [], [], [], []
    partition_name = nc.partition_id_tensor.name if nc.partition_id_tensor else None
    for alloc in nc.m.functions[0].allocations:
        if not isinstance(alloc, mybir.MemoryLocationSet):
            continue
        name = alloc.memorylocations[0].name
        if alloc.kind == "ExternalInput":
            if name != partition_name:
                in_names.append(name)
        elif alloc.kind == "ExternalOutput":
            shape = tuple(alloc.tensor_shape)
            dtype = mybir.dt.np(alloc.dtype)
            out_names.append(name)
            out_avals.append(jax.core.ShapedArray(shape, dtype))
            zero_outs.append(np.zeros(shape, dtype))
    n_params = len(in_names)
    all_in_names = list(in_names) + list(out_names)
    if partition_name is not None:
        all_in_names.append(partition_name)
    donate = tuple(range(n_params, n_params + len(out_names)))

    def _body(*args):
        operands = list(args)
        if partition_name is not None:
            operands.append(partition_id_tensor())
        outs = _bass_exec_p.bind(
            *operands,
            out_avals=tuple(out_avals),
            in_names=tuple(all_in_names),
            out_names=tuple(out_names),
            lowering_input_output_aliases=(),
            sim_require_finite=True,
            sim_require_nnan=True,
            nc=nc,
        )
        return tuple(outs)

    devices = jax.devices()[:NCORES]
    mesh = Mesh(np.asarray(devices), ("core",))
    in_specs = (PartitionSpec("core"),) * (n_params + len(out_names))
    out_specs = (PartitionSpec("core"),) * len(out_names)
    sharded = jax.jit(
        shard_map(
            _body, mesh=mesh, in_specs=in_specs, out_specs=out_specs, check_rep=False
        ),
        donate_argnums=donate,
        keep_unused=True,
    )

    def run(in_maps):
        concat_in = [
            np.concatenate([np.asarray(in_maps[c][name]) for c in range(NCORES)], axis=0)
            for name in in_names
        ]
        concat_zeros = [
            np.zeros((NCORES * z.shape[0], *z.shape[1:]), z.dtype) for z in zero_outs
        ]
        out_arrs = sharded(*concat_in, *concat_zeros)
        return [
            {
                name: np.asarray(out_arrs[i]).reshape(NCORES, *out_avals[i].shape)[c]
                for i, name in enumerate(out_names)
            }
            for c in range(NCORES)
        ]

    _NC_CACHE["runner"] = run
    return run


def kernel(xyz1, xyz2):
    global LAST_RESULTS

    xyz1 = np.asarray(xyz1)
    xyz2 = np.asarray(xyz2)

    in_maps = []
    for i in range(NCORES):
        b, h = divmod(i, 2)
        p = xyz1[b, h * NLOC:(h + 1) * NLOC]
        q = xyz2[b]
        aug1, aug2 = _make_augs(p, q)
        in_maps.append({"aug1": aug1, "aug2": aug2})

    results = _get_runner()(in_maps)
    LAST_RESULTS = results

    tot_row = 0.0
    colvecs = []
    for i in range(NCORES):
        r = results[i]
        tot_row += np.asarray(r["rowmin"], dtype=np.float64).sum()
        cm = np.asarray(r["colmin"], dtype=np.float64)  # [m%128, m//128]
        colvecs.append(cm.T.reshape(-1))                # index by m
    tot_col = 0.0
    for b in range(B):
        tot_col += np.minimum(colvecs[2 * b], colvecs[2 * b + 1]).sum()

    val = tot_row / (B * N) + tot_col / (B * M)
    return np.asarray(val, dtype=np.float32)



# revision 3
# speedup vs baseline: 3.1425x; 3.1425x over previous
"""Chamfer distance (squared-L2) kernel for 8 Trainium2 NeuronCores.

Problem: xyz1 (4, 8192, 3) f32, xyz2 (4, 8192, 3) f32.
  d[b,n,m] = ||p_n - q_m||^2 ; out = mean_n(min_m d) + mean_m(min_n d)  (scalar f32)

Sharding: 8 cores = 4 batches x 2-way split of N.  Each core handles a
(4096 x 8192) block of the distance matrix: full row-mins for its 4096 rows
plus partial column-mins (min-combined across the 2 row-shards on host).

The whole warm-path cost in this environment is the axon tunnel round trip
(~70 ms) plus transfer bytes (~7 ms/MB up, ~15 ms/MB down); on-device time
(<1 ms) is invisible inside that window.  The design is therefore built
around ONE round trip and minimal bytes:

  - Inputs per core are just the raw points, pre-transposed on host:
      a = -2 * xyz1-shard^T  (3, 4096) f32,  q = xyz2[b]^T  (3, 8192) f32
    (1.2 MB total across 8 cores vs 4.7 MB for host-built bf16-split
    operands).
  - The augmented matmul operands are built ON DEVICE: K=9 fp32 rows
      lhsT = [a_x a_y a_z | p_x^2 p_y^2 p_z^2 | 1 1 1]
      rhs  = [q_x q_y q_z | 1     1     1     | q_x^2 q_y^2 q_z^2]
    so lhsT.T @ rhs = ||p||^2 + ||q||^2 - 2 p.q exactly in fp32; the
    squares come from one ACT Square op per side (scale=0.5 turns -2p
    into p^2), the ones rows from memset.  fp32 matmul streams at 4
    cyc/col -> ~0.8 ms/core of PE time, hidden inside the round trip.
  - ScalarE copies PSUM distance tiles to SBUF narrowed to bf16
    (round-to-nearest noise on the mins averages out over 32k rows/cols).
  - VectorE tensor_reduce(min) per 128-row tile -> row-mins; a bf16
    running accumulator updated with tensor_tensor(min) -> column-mins.
  - PE transposes the (128, 8192) column-min accumulator in 128x128
    blocks; VectorE segmented min-reduces produce per-column mins.
  - ONE output per core: (128, 96) bf16 = [rowmin (128,32) | colmin
    (128,64)] (bf16 is lossless here: all values are mins of bf16
    numbers).  A single output array means a single D2H fetch, which
    pipelines with the upload + execute into one tunnel round trip.

Host combine (~1 ms): sum row-mins, pairwise-min + sum column-mins.
"""

import os
import numpy as np
import ml_dtypes

B = 4
N = 8192
M = 8192
NCORES = 8
NLOC = N // 2            # 4096 rows per core
P = 128                  # partitions
NT = NLOC // P           # 32 n-tiles
CHUNK = 2048             # columns per PSUM macro-tile
NCH = M // CHUNK         # 4 chunks
MMF = 512                # matmul free dim (one PSUM bank of fp32)
KAUG = 9                 # augmented contraction size (fp32 rows)
NBLK = M // P            # 64 column blocks of 128 for the final fold
OUTW = NT + NBLK         # 96 output columns: [rowmin | colmin]

BF16 = ml_dtypes.bfloat16

_NC_CACHE = {}


def _build_nc():
    import concourse.bass as bass
    import concourse.mybir as mybir
    import concourse.tile as tile
    import concourse.bacc as bacc
    from concourse.masks import make_identity
    from contextlib import ExitStack

    f32 = mybir.dt.float32
    bf16 = mybir.dt.bfloat16
    MIN = mybir.AluOpType.min
    AXX = mybir.AxisListType.X
    SQUARE = mybir.ActivationFunctionType.Square

    nc = bacc.Bacc(trn_type="TRN2")
    a_d = nc.dram_tensor("a", (3, NLOC), f32, kind="ExternalInput").ap()
    q_d = nc.dram_tensor("q", (3, M), f32, kind="ExternalInput").ap()
    out_d = nc.dram_tensor("out", (P, OUTW), bf16, kind="ExternalOutput").ap()

    with tile.TileContext(nc) as tc, ExitStack() as ctx:
        consts = ctx.enter_context(tc.tile_pool(name="consts", bufs=1))
        accp = ctx.enter_context(tc.tile_pool(name="accp", bufs=1))
        psum = ctx.enter_context(tc.tile_pool(name="psum", bufs=2, space="PSUM"))
        dsb = ctx.enter_context(tc.tile_pool(name="dsb", bufs=3))
        outp = ctx.enter_context(tc.tile_pool(name="outp", bufs=1))

        # augmented matmul operands, built from the raw points.  Compute
        # engines need 32-aligned SBUF partition bases, so squares/ones are
        # produced in partition-0 scratch tiles and DMA'd (no alignment
        # constraint) into the aug rows.
        aug1 = consts.tile([KAUG, NLOC], f32)
        aug2 = consts.tile([KAUG, M], f32)
        sq = consts.tile([3, M], f32)
        ones = consts.tile([3, M], f32)
        nc.sync.dma_start(out=aug1[0:3, :], in_=a_d)
        nc.gpsimd.dma_start(out=aug2[0:3, :], in_=q_d)
        nc.vector.memset(ones, 1.0)
        # p^2 rows: a = -2p, so (0.5*a)^2 = p^2
        nc.scalar.activation(out=sq[:, :NLOC], in_=aug1[0:3, :], func=SQUARE,
                             scale=0.5)
        nc.sync.dma_start(out=aug1[3:6, :], in_=sq[:, :NLOC])
        nc.sync.dma_start(out=aug1[6:9, :], in_=ones[:, :NLOC])
        nc.scalar.activation(out=sq, in_=aug2[0:3, :], func=SQUARE)
        nc.sync.dma_start(out=aug2[6:9, :], in_=sq)
        nc.sync.dma_start(out=aug2[3:6, :], in_=ones)

        ident = consts.tile([P, P], bf16)
        make_identity(nc, ident)

        # column-min accumulator, bf16 (DVE tensor_tensor min runs at
        # 2x_1P for bf16 SBUF operands)
        acc = accp.tile([P, M], bf16)

        rmall = outp.tile([P, NT], f32)
        out_sb = outp.tile([P, OUTW], bf16)

        repeat = int(os.environ.get("CHAMFER_REPEAT", "1"))
        for rep in range(repeat):
          for t in range(NT):
            d = dsb.tile([P, M], bf16, tag="d")
            for c in range(NCH):
                ps = psum.tile([P, CHUNK], f32, tag="ps")
                for j in range(CHUNK // MMF):
                    col = c * CHUNK + j * MMF
                    nc.tensor.matmul(
                        ps[:, j * MMF:(j + 1) * MMF],
                        aug1[:, t * P:(t + 1) * P],
                        aug2[:, col:col + MMF],
                        start=True,
                        stop=True,
                    )
                # ACT copies + narrows to bf16 (min results only need bf16:
                # round-to-nearest noise averages out over 32k rows/cols)
                nc.scalar.copy(out=d[:, c * CHUNK:(c + 1) * CHUNK], in_=ps)

            nc.vector.tensor_reduce(
                out=rmall[:, t:t + 1], in_=d, axis=AXX, op=MIN
            )
            if t == 0 and rep == 0:
                nc.vector.tensor_copy(out=acc, in_=d)
            else:
                nc.vector.tensor_tensor(out=acc, in0=d, in1=acc, op=MIN)

        # row-min columns of the output (bf16 narrowing is lossless: the
        # f32 values are mins of bf16 numbers)
        nc.scalar.copy(out=out_sb[:, :NT], in_=rmall)

        # fold the column-min accumulator over the partition axis:
        # PE-transpose 128x128 bf16 blocks into PSUM, then segmented
        # min-reduce straight into the output tile.
        TGRP = 8
        for g in range(NBLK // TGRP):
            psT = psum.tile([P, TGRP * P], bf16, tag="ps")
            for j in range(TGRP):
                k = g * TGRP + j
                nc.tensor.transpose(
                    psT[:, j * P:(j + 1) * P], acc[:, k * P:(k + 1) * P], ident
                )
            seg = psT.rearrange("p (j x) -> p j x", x=P)
            nc.vector.tensor_reduce(
                out=out_sb[:, NT + g * TGRP:NT + (g + 1) * TGRP],
                in_=seg, axis=AXX, op=MIN,
            )

        nc.sync.dma_start(out=out_d, in_=out_sb)
    nc.compile()
    return nc


def _get_runner():
    """Build (once) a cached jitted SPMD executor for the bass program.

    Mirrors concourse.bass2jax.run_bass_via_pjrt's multi-core path, but
    caches the jitted callable so repeat kernel() calls skip
    retrace/recompile, and returns the single global output array via one
    np.asarray call -- upload, execute and fetch then pipeline into a
    single tunnel round trip.
    """
    if "runner" in _NC_CACHE:
        return _NC_CACHE["runner"]

    import jax
    import concourse.mybir as mybir
    from jax.experimental.shard_map import shard_map
    from jax.sharding import Mesh, PartitionSpec
    from concourse.bass2jax import (
        install_neuronx_cc_hook,
        partition_id_tensor,
        _bass_exec_p,
    )

    install_neuronx_cc_hook()
    nc = _build_nc()

    in_names, out_names, out_avals, zero_outs = [], [], [], []
    partition_name = nc.partition_id_tensor.name if nc.partition_id_tensor else None
    for alloc in nc.m.functions[0].allocations:
        if not isinstance(alloc, mybir.MemoryLocationSet):
            continue
        name = alloc.memorylocations[0].name
        if alloc.kind == "ExternalInput":
            if name != partition_name:
                in_names.append(name)
        elif alloc.kind == "ExternalOutput":
            shape = tuple(alloc.tensor_shape)
            dtype = mybir.dt.np(alloc.dtype)
            out_names.append(name)
            out_avals.append(jax.core.ShapedArray(shape, dtype))
            zero_outs.append(np.zeros((NCORES * shape[0], *shape[1:]), dtype))
    n_params = len(in_names)
    all_in_names = list(in_names) + list(out_names)
    if partition_name is not None:
        all_in_names.append(partition_name)
    donate = tuple(range(n_params, n_params + len(out_names)))

    def _body(*args):
        operands = list(args)
        if partition_name is not None:
            operands.append(partition_id_tensor())
        outs = _bass_exec_p.bind(
            *operands,
            out_avals=tuple(out_avals),
            in_names=tuple(all_in_names),
            out_names=tuple(out_names),
            lowering_input_output_aliases=(),
            sim_require_finite=True,
            sim_require_nnan=True,
            nc=nc,
        )
        return tuple(outs)

    devices = jax.devices()[:NCORES]
    mesh = Mesh(np.asarray(devices), ("core",))
    in_specs = (PartitionSpec("core"),) * (n_params + len(out_names))
    out_specs = (PartitionSpec("core"),) * len(out_names)
    sharded = jax.jit(
        shard_map(
            _body, mesh=mesh, in_specs=in_specs, out_specs=out_specs, check_rep=False
        ),
        donate_argnums=donate,
        keep_unused=True,
    )

    def run(a_cat, q_cat):
        out = sharded(a_cat, q_cat, *[z for z in zero_outs])
        # single fetch of the single global output: one round trip total
        return np.asarray(out[0])

    _NC_CACHE["runner"] = run
    return run


def kernel(xyz1, xyz2):
    xyz1 = np.asarray(xyz1)
    xyz2 = np.asarray(xyz2)

    # per-core inputs, concatenated along axis 0 for shard_map:
    # core i = (batch b = i//2, half h = i%2)
    a_cat = np.empty((NCORES * 3, NLOC), np.float32)
    q_cat = np.empty((NCORES * 3, M), np.float32)
    for i in range(NCORES):
        b, h = divmod(i, 2)
        np.multiply(xyz1[b, h * NLOC:(h + 1) * NLOC].T, -2.0,
                    out=a_cat[i * 3:(i + 1) * 3])
        q_cat[i * 3:(i + 1) * 3] = xyz2[b].T

    out = _get_runner()(a_cat, q_cat)  # (NCORES*128, 96) bf16

    out64 = out.astype(np.float64).reshape(NCORES, P, OUTW)
    tot_row = out64[:, :, :NT].sum()
    # colmin layout: column m = k*128 + p  ->  [p, NT + k]
    colvecs = out64[:, :, NT:].transpose(0, 2, 1).reshape(NCORES, -1)
    tot_col = np.minimum(colvecs[0::2], colvecs[1::2]).sum()

    val = tot_row / (B * N) + tot_col / (B * M)
    return np.asarray(val, dtype=np.float32)


# revision 4
# speedup vs baseline: 3.4764x; 1.1063x over previous
"""Chamfer distance (squared-L2) kernel for Trainium2 NeuronCores (axon).

Problem: xyz1 (4, 8192, 3) f32, xyz2 (4, 8192, 3) f32.
  d[b,n,m] = ||p_n - q_m||^2 ; out = mean_n(min_m d) + mean_m(min_n d)  (scalar f32)

The warm-path cost in this environment is one axon tunnel round trip
(~60-90 ms) plus transfer bytes (~10-25 ms/MB); on-device compute (~1.5 ms)
is invisible inside that window.  The design therefore minimizes round
trips and bytes:

  - ONE round trip: a single cached jitted shard_map call with numpy args
    and a SINGLE output array fetched by a single np.asarray -- upload,
    execute and fetch all pipeline into one tunnel round trip.
  - Minimal upload: data-parallel over B on 4 cores, one full batch per
    core, so every point is uploaded exactly once (786 KB total: per core
    a = -2*xyz1[b]^T and q = xyz2[b]^T, both (3, 8192) f32).  Splitting N
    further across 8 cores would duplicate xyz2 (+50% bytes) for ~0.8 ms
    of hidden device time -- a strictly worse trade here.
  - Minimal fetch: per-core column-mins are COMPLETE (the core sees all
    rows of its batch), so one (128, 128) bf16 output per core:
    [row-mins (128,64) | col-mins (128,64)], 131 KB total.  bf16 is
    lossless for these values (they are mins of bf16 numbers).

Device kernel (per core, batch b):
  - Augmented K=9 fp32 matmul emits complete squared-distance tiles:
      lhsT = [a_x a_y a_z | p_x^2 p_y^2 p_z^2 | 1 1 1]      (9, 8192)
      rhs  = [q_x q_y q_z | 1     1     1     | q_x^2 q_y^2 q_z^2]
    so lhsT.T @ rhs = ||p||^2 + ||q||^2 - 2 p.q exactly in fp32.  The
    operand rows are built on device: squares via one ACT Square per side
    (scale=0.5 turns a=-2p into p^2), ones via memset; compute engines
    need 32-aligned SBUF partition bases, so these go through a
    partition-0 scratch tile and are DMA'd (no alignment constraint) into
    rows 3..8.  fp32 matmul streams at 4 cyc/col -> ~1.5 ms PE time.
  - ScalarE copies PSUM distance tiles to SBUF narrowed to bf16
    (round-to-nearest noise on the mins averages out over 32k rows/cols).
  - VectorE tensor_reduce(min) per 128-row tile -> row-mins; a bf16
    running accumulator updated with tensor_tensor(min) -> column-mins.
  - PE transposes the (128, 8192) column-min accumulator in 128x128
    blocks; VectorE segmented min-reduces produce per-column mins.

Host (~1 ms): transpose/scale views into the concat upload buffers; sum
the fetched mins in f64.
"""

import os
import numpy as np
import ml_dtypes

B = 4
N = 8192
M = 8192
CORES = 4                # data-parallel over B: one batch per core
P = 128                  # partitions
NT = N // P              # 64 n-tiles
CHUNK = 2048             # columns per PSUM macro-tile
NCH = M // CHUNK         # 4 chunks
MMF = 512                # matmul free dim (one PSUM bank of fp32)
KAUG = 9                 # augmented contraction size (fp32 rows)
NBLK = M // P            # 64 column blocks of 128 for the final fold
OUTW = NT + NBLK         # 128 output columns: [rowmin | colmin]

BF16 = ml_dtypes.bfloat16

_NC_CACHE = {}


def _build_nc():
    import concourse.bass as bass
    import concourse.mybir as mybir
    import concourse.tile as tile
    import concourse.bacc as bacc
    from concourse.masks import make_identity
    from contextlib import ExitStack

    f32 = mybir.dt.float32
    bf16 = mybir.dt.bfloat16
    MIN = mybir.AluOpType.min
    AXX = mybir.AxisListType.X
    SQUARE = mybir.ActivationFunctionType.Square

    nc = bacc.Bacc(trn_type="TRN2")
    a_d = nc.dram_tensor("a", (3, N), f32, kind="ExternalInput").ap()
    q_d = nc.dram_tensor("q", (3, M), f32, kind="ExternalInput").ap()
    out_d = nc.dram_tensor("out", (P, OUTW), bf16, kind="ExternalOutput").ap()

    with tile.TileContext(nc) as tc, ExitStack() as ctx:
        consts = ctx.enter_context(tc.tile_pool(name="consts", bufs=1))
        accp = ctx.enter_context(tc.tile_pool(name="accp", bufs=1))
        psum = ctx.enter_context(tc.tile_pool(name="psum", bufs=2, space="PSUM"))
        dsb = ctx.enter_context(tc.tile_pool(name="dsb", bufs=3))
        outp = ctx.enter_context(tc.tile_pool(name="outp", bufs=1))

        # augmented matmul operands, built from the raw points via one
        # shared partition-0 scratch tile (see module docstring)
        aug1 = consts.tile([KAUG, N], f32)
        aug2 = consts.tile([KAUG, M], f32)
        scr = consts.tile([3, M], f32)
        nc.sync.dma_start(out=aug1[0:3, :], in_=a_d)
        nc.gpsimd.dma_start(out=aug2[0:3, :], in_=q_d)
        nc.vector.memset(scr, 1.0)
        nc.sync.dma_start(out=aug1[6:9, :], in_=scr)
        nc.sync.dma_start(out=aug2[3:6, :], in_=scr)
        # p^2 rows: a = -2p, so (0.5*a)^2 = p^2
        nc.scalar.activation(out=scr, in_=aug1[0:3, :], func=SQUARE, scale=0.5)
        nc.sync.dma_start(out=aug1[3:6, :], in_=scr)
        nc.scalar.activation(out=scr, in_=aug2[0:3, :], func=SQUARE)
        nc.sync.dma_start(out=aug2[6:9, :], in_=scr)

        ident = consts.tile([P, P], bf16)
        make_identity(nc, ident)

        # column-min accumulator, bf16 (DVE tensor_tensor min runs at
        # 2x_1P for bf16 SBUF operands)
        acc = accp.tile([P, M], bf16)

        rmall = outp.tile([P, NT], f32)
        out_sb = outp.tile([P, OUTW], bf16)

        repeat = int(os.environ.get("CHAMFER_REPEAT", "1"))
        for rep in range(repeat):
          for t in range(NT):
            d = dsb.tile([P, M], bf16, tag="d")
            for c in range(NCH):
                ps = psum.tile([P, CHUNK], f32, tag="ps")
                for j in range(CHUNK // MMF):
                    col = c * CHUNK + j * MMF
                    nc.tensor.matmul(
                        ps[:, j * MMF:(j + 1) * MMF],
                        aug1[:, t * P:(t + 1) * P],
                        aug2[:, col:col + MMF],
                        start=True,
                        stop=True,
                    )
                # ACT copies + narrows to bf16 (min results only need bf16:
                # round-to-nearest noise averages out over 32k rows/cols)
                nc.scalar.copy(out=d[:, c * CHUNK:(c + 1) * CHUNK], in_=ps)

            nc.vector.tensor_reduce(
                out=rmall[:, t:t + 1], in_=d, axis=AXX, op=MIN
            )
            if t == 0 and rep == 0:
                nc.vector.tensor_copy(out=acc, in_=d)
            else:
                nc.vector.tensor_tensor(out=acc, in0=d, in1=acc, op=MIN)

        # row-min columns of the output (bf16 narrowing is lossless: the
        # f32 values are mins of bf16 numbers)
        nc.scalar.copy(out=out_sb[:, :NT], in_=rmall)

        # fold the column-min accumulator over the partition axis:
        # PE-transpose 128x128 bf16 blocks into PSUM, then segmented
        # min-reduce straight into the output tile.
        TGRP = 8
        for g in range(NBLK // TGRP):
            psT = psum.tile([P, TGRP * P], bf16, tag="ps")
            for j in range(TGRP):
                k = g * TGRP + j
                nc.tensor.transpose(
                    psT[:, j * P:(j + 1) * P], acc[:, k * P:(k + 1) * P], ident
                )
            seg = psT.rearrange("p (j x) -> p j x", x=P)
            nc.vector.tensor_reduce(
                out=out_sb[:, NT + g * TGRP:NT + (g + 1) * TGRP],
                in_=seg, axis=AXX, op=MIN,
            )

        nc.sync.dma_start(out=out_d, in_=out_sb)
    nc.compile()
    return nc


def _get_runner():
    """Build (once) a cached jitted SPMD executor for the bass program.

    Mirrors concourse.bass2jax.run_bass_via_pjrt's multi-core path, but
    caches the jitted callable so repeat kernel() calls skip
    retrace/recompile, and fetches the single global output with one
    np.asarray call -- upload, execute and fetch then pipeline into a
    single tunnel round trip.
    """
    if "runner" in _NC_CACHE:
        return _NC_CACHE["runner"]

    import jax
    import concourse.mybir as mybir
    from jax.experimental.shard_map import shard_map
    from jax.sharding import Mesh, PartitionSpec
    from concourse.bass2jax import (
        install_neuronx_cc_hook,
        partition_id_tensor,
        _bass_exec_p,
    )

    install_neuronx_cc_hook()
    nc = _build_nc()

    in_names, out_names, out_avals, zero_outs = [], [], [], []
    partition_name = nc.partition_id_tensor.name if nc.partition_id_tensor else None
    for alloc in nc.m.functions[0].allocations:
        if not isinstance(alloc, mybir.MemoryLocationSet):
            continue
        name = alloc.memorylocations[0].name
        if alloc.kind == "ExternalInput":
            if name != partition_name:
                in_names.append(name)
        elif alloc.kind == "ExternalOutput":
            shape = tuple(alloc.tensor_shape)
            dtype = mybir.dt.np(alloc.dtype)
            out_names.append(name)
            out_avals.append(jax.core.ShapedArray(shape, dtype))
            zero_outs.append(np.zeros((CORES * shape[0], *shape[1:]), dtype))
    n_params = len(in_names)
    all_in_names = list(in_names) + list(out_names)
    if partition_name is not None:
        all_in_names.append(partition_name)
    donate = tuple(range(n_params, n_params + len(out_names)))

    def _body(*args):
        operands = list(args)
        if partition_name is not None:
            operands.append(partition_id_tensor())
        outs = _bass_exec_p.bind(
            *operands,
            out_avals=tuple(out_avals),
            in_names=tuple(all_in_names),
            out_names=tuple(out_names),
            lowering_input_output_aliases=(),
            sim_require_finite=True,
            sim_require_nnan=True,
            nc=nc,
        )
        return tuple(outs)

    devices = jax.devices()[:CORES]
    mesh = Mesh(np.asarray(devices), ("core",))
    in_specs = (PartitionSpec("core"),) * (n_params + len(out_names))
    out_specs = (PartitionSpec("core"),) * len(out_names)
    sharded = jax.jit(
        shard_map(
            _body, mesh=mesh, in_specs=in_specs, out_specs=out_specs, check_rep=False
        ),
        donate_argnums=donate,
        keep_unused=True,
    )

    def run(a_cat, q_cat):
        out = sharded(a_cat, q_cat, *zero_outs)
        # single fetch of the single global output: one round trip total
        return np.asarray(out[0])

    _NC_CACHE["runner"] = run
    return run


def kernel(xyz1, xyz2):
    xyz1 = np.asarray(xyz1)
    xyz2 = np.asarray(xyz2)

    # per-core (= per-batch) inputs, concatenated along axis 0 for shard_map
    a_cat = np.empty((CORES * 3, N), np.float32)
    q_cat = np.empty((CORES * 3, M), np.float32)
    for b in range(CORES):
        np.multiply(xyz1[b].T, -2.0, out=a_cat[b * 3:(b + 1) * 3])
        q_cat[b * 3:(b + 1) * 3] = xyz2[b].T

    out = _get_runner()(a_cat, q_cat)  # (CORES*128, 128) bf16

    out64 = out.astype(np.float64).reshape(CORES, P, OUTW)
    tot_row = out64[:, :, :NT].sum()
    tot_col = out64[:, :, NT:].sum()

    val = tot_row / (B * N) + tot_col / (B * M)
    return np.asarray(val, dtype=np.float32)


# revision 8
# speedup vs baseline: 4.5299x; 1.3030x over previous
"""Chamfer distance (squared-L2) kernel for Trainium2 NeuronCores (axon).

Problem: xyz1 (4, 8192, 3) f32, xyz2 (4, 8192, 3) f32.
  d[b,n,m] = ||p_n - q_m||^2 ; out = mean_n(min_m d) + mean_m(min_n d)  (scalar f32)

The warm-path cost in this environment is one axon tunnel round trip
(~60-90 ms) plus transfer bytes (~10-25 ms/MB); on-device compute (~1.5 ms)
is invisible inside that window.  The design therefore minimizes round
trips and bytes:

  - ONE round trip: a single cached jitted shard_map call with numpy args
    and a SINGLE output array fetched by a single np.asarray -- upload,
    execute and fetch all pipeline into one tunnel round trip.
  - Minimal upload: data-parallel over B on 4 cores, one full batch per
    core, so every point is uploaded exactly once (786 KB total: per core
    a = -2*xyz1[b]^T and q = xyz2[b]^T, both (3, 8192) f32).  Splitting N
    further across 8 cores would duplicate xyz2 (+50% bytes) for ~0.8 ms
    of hidden device time -- a strictly worse trade here.
  - Minimal fetch: per-core column-mins are COMPLETE (the core sees all
    rows of its batch), so one (128, 128) bf16 output per core:
    [row-mins (128,64) | col-mins (128,64)], 131 KB total.  bf16 is
    lossless for these values (they are mins of bf16 numbers).

Device kernel (per core, batch b):
  - Augmented K=9 fp32 matmul emits complete squared-distance tiles:
      lhsT = [a_x a_y a_z | p_x^2 p_y^2 p_z^2 | 1 1 1]      (9, 8192)
      rhs  = [q_x q_y q_z | 1     1     1     | q_x^2 q_y^2 q_z^2]
    so lhsT.T @ rhs = ||p||^2 + ||q||^2 - 2 p.q exactly in fp32.  The
    operand rows are built on device: squares via one ACT Square per side
    (scale=0.5 turns a=-2p into p^2), ones via memset; compute engines
    need 32-aligned SBUF partition bases, so these go through a
    partition-0 scratch tile and are DMA'd (no alignment constraint) into
    rows 3..8.  fp32 matmul streams at 4 cyc/col -> ~1.5 ms PE time.
  - ScalarE copies PSUM distance tiles to SBUF narrowed to bf16
    (round-to-nearest noise on the mins averages out over 32k rows/cols).
  - VectorE tensor_reduce(min) per 128-row tile -> row-mins; a bf16
    running accumulator updated with tensor_tensor(min) -> column-mins.
  - PE transposes the (128, 8192) column-min accumulator in 128x128
    blocks; VectorE segmented min-reduces produce per-column mins.

Host (~1 ms): transpose/scale views into the concat upload buffers; sum
the fetched mins in f64.
"""

import os
import numpy as np
import ml_dtypes

B = 4
N = 8192
M = 8192
CORES = 4                # data-parallel over B: one batch per core
P = 128                  # partitions
NT = N // P              # 64 n-tiles
CHUNK = 2048             # columns per PSUM macro-tile
NCH = M // CHUNK         # 4 chunks
MMF = 512                # matmul free dim (one PSUM bank of fp32)
KAUG = 9                 # augmented contraction size (fp32 rows)
NBLK = M // P            # 64 column blocks of 128 for the final fold
OUTW = NT + NBLK         # 128 output columns: [rowmin | colmin]

BF16 = ml_dtypes.bfloat16

_NC_CACHE = {}


def _build_nc():
    import concourse.bass as bass
    import concourse.mybir as mybir
    import concourse.tile as tile
    import concourse.bacc as bacc
    from concourse.masks import make_identity
    from contextlib import ExitStack

    f32 = mybir.dt.float32
    bf16 = mybir.dt.bfloat16
    MIN = mybir.AluOpType.min
    AXX = mybir.AxisListType.X
    SQUARE = mybir.ActivationFunctionType.Square

    ADD = mybir.AluOpType.add

    nc = bacc.Bacc(trn_type="TRN2")
    # single input tensor: rows 0:3 = a = -2*xyz1[b]^T, rows 3:6 = xyz2[b]^T
    pts_d = nc.dram_tensor("pts", (6, N), f32, kind="ExternalInput").ap()
    # single tiny output: [sum of row-mins, sum of col-mins]
    out_d = nc.dram_tensor("out", (1, 2), f32, kind="ExternalOutput").ap()

    with tile.TileContext(nc) as tc, ExitStack() as ctx:
        consts = ctx.enter_context(tc.tile_pool(name="consts", bufs=1))
        accp = ctx.enter_context(tc.tile_pool(name="accp", bufs=1))
        psum = ctx.enter_context(tc.tile_pool(name="psum", bufs=2, space="PSUM"))
        dsb = ctx.enter_context(tc.tile_pool(name="dsb", bufs=3))
        outp = ctx.enter_context(tc.tile_pool(name="outp", bufs=1))

        # augmented matmul operands, built from the raw points via one
        # shared partition-0 scratch tile (see module docstring)
        aug1 = consts.tile([KAUG, N], f32)
        aug2 = consts.tile([KAUG, M], f32)
        scr = consts.tile([3, M], f32)
        nc.sync.dma_start(out=aug1[0:3, :], in_=pts_d[0:3, :])
        nc.gpsimd.dma_start(out=aug2[0:3, :], in_=pts_d[3:6, :])
        nc.vector.memset(scr, 1.0)
        nc.sync.dma_start(out=aug1[6:9, :], in_=scr)
        nc.sync.dma_start(out=aug2[3:6, :], in_=scr)
        # p^2 rows: a = -2p, so (0.5*a)^2 = p^2
        nc.scalar.activation(out=scr, in_=aug1[0:3, :], func=SQUARE, scale=0.5)
        nc.sync.dma_start(out=aug1[3:6, :], in_=scr)
        nc.scalar.activation(out=scr, in_=aug2[0:3, :], func=SQUARE)
        nc.sync.dma_start(out=aug2[6:9, :], in_=scr)

        ident = consts.tile([P, P], bf16)
        make_identity(nc, ident)

        # column-min accumulator, bf16 (DVE tensor_tensor min runs at
        # 2x_1P for bf16 SBUF operands)
        acc = accp.tile([P, M], bf16)

        rmall = outp.tile([P, NT], f32)
        out_sb = outp.tile([P, OUTW], bf16)

        repeat = int(os.environ.get("CHAMFER_REPEAT", "1"))
        for rep in range(repeat):
          for t in range(NT):
            d = dsb.tile([P, M], bf16, tag="d")
            for c in range(NCH):
                ps = psum.tile([P, CHUNK], f32, tag="ps")
                for j in range(CHUNK // MMF):
                    col = c * CHUNK + j * MMF
                    nc.tensor.matmul(
                        ps[:, j * MMF:(j + 1) * MMF],
                        aug1[:, t * P:(t + 1) * P],
                        aug2[:, col:col + MMF],
                        start=True,
                        stop=True,
                    )
                # ACT copies + narrows to bf16 (min results only need bf16:
                # round-to-nearest noise averages out over 32k rows/cols)
                nc.scalar.copy(out=d[:, c * CHUNK:(c + 1) * CHUNK], in_=ps)

            nc.vector.tensor_reduce(
                out=rmall[:, t:t + 1], in_=d, axis=AXX, op=MIN
            )
            if t == 0 and rep == 0:
                nc.vector.tensor_copy(out=acc, in_=d)
            else:
                nc.vector.tensor_tensor(out=acc, in0=d, in1=acc, op=MIN)

        # row-min columns of the output (bf16 narrowing is lossless: the
        # f32 values are mins of bf16 numbers)
        nc.scalar.copy(out=out_sb[:, :NT], in_=rmall)

        # fold the column-min accumulator over the partition axis:
        # PE-transpose 128x128 bf16 blocks into PSUM, then segmented
        # min-reduce straight into the output tile.
        TGRP = 8
        for g in range(NBLK // TGRP):
            psT = psum.tile([P, TGRP * P], bf16, tag="ps")
            for j in range(TGRP):
                k = g * TGRP + j
                nc.tensor.transpose(
                    psT[:, j * P:(j + 1) * P], acc[:, k * P:(k + 1) * P], ident
                )
            seg = psT.rearrange("p (j x) -> p j x", x=P)
            nc.vector.tensor_reduce(
                out=out_sb[:, NT + g * TGRP:NT + (g + 1) * TGRP],
                in_=seg, axis=AXX, op=MIN,
            )

        # fold out_sb ([rowmin | colmin], (128, 128) bf16) to two scalars:
        # ones-matmul sums the partition axis into PSUM (exact: bf16 values
        # accumulated in f32), then a segmented add-reduce sums each half.
        ones128 = consts.tile([P, 1], bf16)
        nc.vector.memset(ones128, 1.0)
        psS = psum.tile([1, OUTW], f32, tag="ps")
        nc.tensor.matmul(psS, ones128, out_sb, start=True, stop=True)
        segS = psS.rearrange("p (j x) -> p j x", x=NT)
        out_fin = outp.tile([1, 2], f32)
        nc.vector.tensor_reduce(out=out_fin, in_=segS, axis=AXX, op=ADD)
        nc.sync.dma_start(out=out_d, in_=out_fin)
    nc.compile()
    return nc


def _get_runner():
    """Build (once) a cached jitted SPMD executor for the bass program.

    Mirrors concourse.bass2jax.run_bass_via_pjrt's multi-core path, but
    caches the jitted callable so repeat kernel() calls skip
    retrace/recompile, and fetches the single global output with one
    np.asarray call -- upload, execute and fetch then pipeline into a
    single tunnel round trip.
    """
    if "runner" in _NC_CACHE:
        return _NC_CACHE["runner"]

    import jax
    import concourse.mybir as mybir
    from jax.experimental.shard_map import shard_map
    from jax.sharding import Mesh, PartitionSpec
    from concourse.bass2jax import (
        install_neuronx_cc_hook,
        partition_id_tensor,
        _bass_exec_p,
    )

    install_neuronx_cc_hook()
    nc = _build_nc()

    in_names, out_names, out_avals, zero_outs = [], [], [], []
    partition_name = nc.partition_id_tensor.name if nc.partition_id_tensor else None
    for alloc in nc.m.functions[0].allocations:
        if not isinstance(alloc, mybir.MemoryLocationSet):
            continue
        name = alloc.memorylocations[0].name
        if alloc.kind == "ExternalInput":
            if name != partition_name:
                in_names.append(name)
        elif alloc.kind == "ExternalOutput":
            shape = tuple(alloc.tensor_shape)
            dtype = mybir.dt.np(alloc.dtype)
            out_names.append(name)
            out_avals.append(jax.core.ShapedArray(shape, dtype))
            zero_outs.append(np.zeros((CORES * shape[0], *shape[1:]), dtype))
    n_params = len(in_names)
    all_in_names = list(in_names) + list(out_names)
    if partition_name is not None:
        all_in_names.append(partition_name)
    donate = tuple(range(n_params, n_params + len(out_names)))

    def _body(*args):
        operands = list(args)
        if partition_name is not None:
            operands.append(partition_id_tensor())
        outs = _bass_exec_p.bind(
            *operands,
            out_avals=tuple(out_avals),
            in_names=tuple(all_in_names),
            out_names=tuple(out_names),
            lowering_input_output_aliases=(),
            sim_require_finite=True,
            sim_require_nnan=True,
            nc=nc,
        )
        return tuple(outs)

    devices = jax.devices()[:CORES]
    mesh = Mesh(np.asarray(devices), ("core",))
    in_specs = (PartitionSpec("core"),) * (n_params + len(out_names))
    out_specs = (PartitionSpec("core"),) * len(out_names)
    sharded = jax.jit(
        shard_map(
            _body, mesh=mesh, in_specs=in_specs, out_specs=out_specs, check_rep=False
        ),
        donate_argnums=donate,
        keep_unused=True,
    )

    def run(pts_cat):
        out = sharded(pts_cat, *zero_outs)
        # single fetch of the single global output: one round trip total
        return np.asarray(out[0])

    _NC_CACHE["runner"] = run
    return run


def kernel(xyz1, xyz2):
    xyz1 = np.asarray(xyz1)
    xyz2 = np.asarray(xyz2)

    # per-core (= per-batch) input, concatenated along axis 0 for shard_map:
    # rows 0:3 = -2*xyz1[b]^T, rows 3:6 = xyz2[b]^T
    pts_cat = np.empty((CORES * 6, N), np.float32)
    for b in range(CORES):
        np.multiply(xyz1[b].T, -2.0, out=pts_cat[b * 6:b * 6 + 3])
        pts_cat[b * 6 + 3:b * 6 + 6] = xyz2[b].T

    out = _get_runner()(pts_cat)  # (CORES*1, 2) f32

    out64 = out.astype(np.float64)
    val = out64[:, 0].sum() / (B * N) + out64[:, 1].sum() / (B * M)
    return np.asarray(val, dtype=np.float32)
